# revision 4
# baseline (speedup 1.0000x reference)
"""Causal single-head attention (B=16, T=2048, C=HEAD=384) on 8 trn2 cores.

Sharding: data-parallel over batch. Each core gets 2 batch elements and
runs the identical Bass program; results are concatenated on the host.

Per-core program (per batch element):
  1. DMA x [T, C] natural -> SBUF, PE-transpose to xT [C, T] (bf16).
  2. qT = Wq^T @ x^T, kT = Wk^T @ x^T   (layout [HEAD, T], head on partitions)
     v  = x @ Wv                        (layout [T, HEAD+1], last col = 1.0)
  3. For each 512-wide query group g, for each causal key block j (128):
     scoresT[j] = kT_blk^T @ qT_grp in PSUM (fp32), add causal mask on
     diagonal blocks, evict with ACT exp(scale * .) to bf16.
  4. PV: out[tq,:] = sum_j weiT[j]^T @ v_ext[j].  The appended ones column
     of v yields the softmax denominator in out[:, C]; normalize the first
     C columns by its reciprocal and DMA out.

No max-subtraction in softmax: scores*scale are ~N(0,1) for these inputs
(max |.| well under 30), so exp cannot overflow fp32 and matches the
reference softmax mathematically.
"""

import os
import sys

import numpy as np

for _p in ("/opt/trn_rl_repo",):
    if os.path.isdir(_p) and _p not in sys.path:
        sys.path.append(_p)

B, T, C = 16, 2048, 384
N_CORES = 8
BPC = B // N_CORES  # batch elements per core
P = 128
NT = T // P  # 16 t-blocks
NCC = C // P  # 3 contraction chunks over C (and over HEAD, since HEAD == C)
GW = 512  # query-group width
NG = T // GW  # 4 query groups
SCALE = float(C) ** -0.5
MASK_BIG = -1e9

# Compute dtype for matmul operands: "bf16" (fastest), "f32r", or "f32".
CDT_NAME = os.environ.get("ATTN_CDT", "bf16")

_cache = {}


def _build(bpc, t, c):
    import concourse.bass as bass  # noqa: F401
    import concourse.mybir as mybir
    from concourse import bacc
    from concourse.masks import make_identity
    from concourse.tile import TileContext

    f32 = mybir.dt.float32
    nt = t // P
    ng = t // GW

    if CDT_NAME == "bf16":
        cdt = mybir.dt.bfloat16
        mm_cast = None
    elif CDT_NAME == "f32r":
        cdt = f32
        mm_cast = mybir.dt.float32r
    else:
        cdt = f32
        mm_cast = None

    def mm(ap):
        return ap.bitcast(mm_cast) if mm_cast is not None else ap

    nc = bacc.Bacc("TRN2", target_bir_lowering=False)

    x_d = nc.declare_dram_parameter("x", [bpc, t, c], f32, isOutput=False)
    w_d = [
        nc.declare_dram_parameter(n, [c, c], f32, isOutput=False)
        for n in ("wq", "wk", "wv")
    ]
    y_d = nc.declare_dram_parameter("y", [bpc, t, c], f32, isOutput=True)

    # bf16 tiles are half size; fp32 variants need smaller pools to fit SBUF
    small = cdt != f32

    with TileContext(nc) as tc:
        with (
            tc.tile_pool(name="singles", bufs=1) as singles,
            tc.tile_pool(name="wstage", bufs=1) as wstage,
            tc.tile_pool(name="xf", bufs=1) as xf_pool,
            tc.tile_pool(name="xT", bufs=2 if small else 1) as xT_pool,
            tc.tile_pool(name="qkT", bufs=2 if small else 1) as qkT_pool,
            tc.tile_pool(name="v", bufs=nt + 4 if small else nt + 1) as v_pool,
            tc.tile_pool(name="wT", bufs=nt + 4 if small else nt + 1) as wT_pool,
            tc.tile_pool(name="outp", bufs=4) as out_pool,
            tc.tile_pool(name="ps_t", bufs=2, space="PSUM") as ps_t,
            tc.tile_pool(name="ps_mm", bufs=6, space="PSUM") as ps_mm,
        ):
            ident = singles.tile([P, P], f32)
            make_identity(nc, ident)

            # mask[s, w] = 0 if w >= s + 3*P else MASK_BIG.
            # Variant v in {0..3} (diagonal offset of s-block j = 4g+v within
            # its query group) is the slice mask[:, 3*P - v*P : 3*P - v*P + GW].
            mw = GW + 3 * P
            mask = singles.tile([P, mw], f32)
            nc.gpsimd.memset(mask, 0.0)
            nc.gpsimd.affine_select(
                out=mask,
                in_=mask,
                compare_op=mybir.AluOpType.is_ge,
                fill=MASK_BIG,
                base=-3 * P,
                pattern=[[1, mw]],
                channel_multiplier=-1,
            )

            # Weights: [c, c] -> [cp, cc, h] (contraction chunk cc on partitions)
            w_sb = []
            for name, wd in zip(("wq", "wk", "wv"), w_d):
                stage = wstage.tile([P, NCC, c], f32, name=f"wst_{name}", tag=f"wst_{name}")
                nc.sync.dma_start(
                    out=stage, in_=wd[:].rearrange("(cc cp) h -> cp cc h", cp=P)
                )
                if cdt == f32:
                    w_sb.append(stage)
                else:
                    wb = singles.tile([P, NCC, c], cdt, name=f"{name}b", tag=f"{name}b")
                    nc.vector.tensor_copy(wb, stage)
                    w_sb.append(wb)

            for b in range(bpc):
                xf = xf_pool.tile([P, nt, c], f32, name="xf", tag="xf")
                nc.sync.dma_start(
                    out=xf, in_=x_d[b].rearrange("(n p) c -> p n c", p=P)
                )

                # ---- transpose x -> xT (c on partitions) ----
                xT = [xT_pool.tile([P, t], cdt, name=f"xT{cc}", tag=f"xT{cc}") for cc in range(NCC)]
                for cc in range(NCC):
                    for n4 in range(nt // 4):
                        pst = ps_t.tile([P, 4, P], f32, name="pst", tag="pst")
                        for k in range(4):
                            n = n4 * 4 + k
                            nc.tensor.transpose(
                                pst[:, k, :], xf[:, n, cc * P : (cc + 1) * P], ident
                            )
                        nc.vector.tensor_copy(
                            xT[cc][:, n4 * 4 * P : (n4 + 1) * 4 * P].rearrange(
                                "p (k q) -> p k q", k=4
                            ),
                            pst,
                        )

                # ---- qT, kT: [HEAD, T] ----
                qT = [qkT_pool.tile([P, t], cdt, name=f"qT{hh}", tag=f"qT{hh}") for hh in range(NCC)]
                kT = [qkT_pool.tile([P, t], cdt, name=f"kT{hh}", tag=f"kT{hh}") for hh in range(NCC)]
                for w, dest in ((w_sb[0], qT), (w_sb[1], kT)):
                    for hh in range(NCC):
                        for tch in range(t // GW):
                            ps = ps_mm.tile([P, GW], f32, name="psmm", tag="mm")
                            for cc in range(NCC):
                                nc.tensor.matmul(
                                    ps,
                                    mm(w[:, cc, hh * P : (hh + 1) * P]),
                                    mm(xT[cc][:, tch * GW : (tch + 1) * GW]),
                                    start=(cc == 0),
                                    stop=(cc == NCC - 1),
                                )
                            nc.vector.tensor_copy(
                                dest[hh][:, tch * GW : (tch + 1) * GW], ps
                            )

                # ---- v natural [T, HEAD+1], last col = 1.0 ----
                v_t = []
                for n in range(nt):
                    vt = v_pool.tile([P, c + 1], cdt, name="vt", tag="v")
                    ps = ps_mm.tile([P, GW], f32, name="psmm", tag="mm")
                    for cc in range(NCC):
                        nc.tensor.matmul(
                            ps[:, :c],
                            mm(xT[cc][:, n * P : (n + 1) * P]),
                            mm(w_sb[2][:, cc, :]),
                            start=(cc == 0),
                            stop=(cc == NCC - 1),
                        )
                    nc.vector.tensor_copy(vt[:, :c], ps[:, :c])
                    nc.vector.memset(vt[:, c : c + 1], 1.0)
                    v_t.append(vt)

                # ---- attention, one 512-wide query group at a time ----
                for g in range(ng):
                    nblk = 4 * g + 4  # causal: s-blocks 0 .. 4g+3
                    wT = []
                    for j in range(nblk):
                        ps = ps_mm.tile([P, GW], f32, name="psmm", tag="mm")
                        for hh in range(NCC):
                            nc.tensor.matmul(
                                ps,
                                mm(kT[hh][:, j * P : (j + 1) * P]),
                                mm(qT[hh][:, g * GW : (g + 1) * GW]),
                                start=(hh == 0),
                                stop=(hh == NCC - 1),
                            )
                        dv = j - 4 * g
                        if dv >= 0:
                            nc.vector.tensor_add(
                                ps, ps, mask[:, 3 * P - dv * P : 3 * P - dv * P + GW]
                            )
                        wt = wT_pool.tile([P, GW], cdt, name="wTt", tag="wT")
                        nc.scalar.activation(
                            out=wt,
                            in_=ps,
                            func=mybir.ActivationFunctionType.Exp,
                            scale=SCALE,
                        )
                        wT.append(wt)

                    for il in range(4):
                        ti = 4 * g + il
                        ps_o = ps_mm.tile([P, c + 1], f32, name="psmo", tag="mm")
                        for j in range(ti + 1):
                            nc.tensor.matmul(
                                ps_o,
                                mm(wT[j][:, il * P : (il + 1) * P]),
                                mm(v_t[j][:]),
                                start=(j == 0),
                                stop=(j == ti),
                            )
                        recip = out_pool.tile([P, 1], f32, name="recip", tag="recip")
                        nc.vector.reciprocal(recip, ps_o[:, c : c + 1])
                        ob = out_pool.tile([P, c], f32, name="ob", tag="ob")
                        nc.vector.tensor_scalar_mul(ob, ps_o[:, :c], recip)
                        nc.sync.dma_start(
                            out=y_d[b, ti * P : (ti + 1) * P, :], in_=ob
                        )

    nc.compile()
    return nc


def _get_nc(bpc, t, c):
    key = (bpc, t, c, CDT_NAME)
    if key not in _cache:
        _cache[key] = _build(bpc, t, c)
    return _cache[key]


def run(x, Wq, Wk, Wv, trace=False):
    """Run on hardware; returns (y, BassKernelResults)."""
    from concourse.bass_utils import run_bass_kernel_spmd

    x = np.ascontiguousarray(np.asarray(x, dtype=np.float32))
    Wq = np.ascontiguousarray(np.asarray(Wq, dtype=np.float32))
    Wk = np.ascontiguousarray(np.asarray(Wk, dtype=np.float32))
    Wv = np.ascontiguousarray(np.asarray(Wv, dtype=np.float32))
    b, t, c = x.shape
    assert b % N_CORES == 0
    bpc = b // N_CORES

    nc = _get_nc(bpc, t, c)
    core_ids = list(range(N_CORES))
    in_maps = [
        {"x": x[i * bpc : (i + 1) * bpc], "wq": Wq, "wk": Wk, "wv": Wv}
        for i in core_ids
    ]
    res = run_bass_kernel_spmd(nc, in_maps, core_ids, trace=trace)
    y = np.concatenate([res.results[i]["y"] for i in core_ids], axis=0)
    return y, res


def kernel(x, Wq, Wk, Wv):
    y, _ = run(x, Wq, Wk, Wv, trace=False)
    return y


# revision 7
# speedup vs baseline: 1.1169x; 1.1169x over previous
"""Causal single-head attention (B=16, T=2048, C=HEAD=384) on 8 trn2 cores.

Sharding: data-parallel over batch. Each core gets 2 batch elements and
runs the identical Bass program; results are concatenated on the host.

Math trick: scores = q @ k^T = x @ (Wq Wk^T) @ x^T. We precompute
TT[c2, c1] = (Wk Wq^T)[c2, c1] once per core, then per batch compute a
single projection kAT = A @ x^T (instead of both q and k), and the
scores matmul streams x^T directly:
    scoresT[s, t] = sum_a kAT[a, s] * xT[a, t].

Per-core program (per batch element):
  1. DMA x [T, C] in 4 chunks -> SBUF, cast bf16 (GpSimd), PE-transpose
     to xT [C, T].
  2. kAT = A @ x^T  (layout [C, T]);  v = x @ Wv  ([T, HEAD+1], last
     col = 1.0).
  3. For each 512-wide query group g, for each causal key block j (128):
     scoresT[j] = kAT_blk^T @ xT_grp in PSUM (fp32; diagonal blocks use a
     narrowed free dim + additive causal mask), evict with ACT
     exp(scale * .) to bf16.
  4. PV: out[tq,:] = sum_j weiT[j]^T @ v_ext[j].  The appended ones
     column of v yields the softmax denominator in out[:, C]; normalize
     by its reciprocal and DMA out.

No max-subtraction in softmax: scores*scale are ~N(0,1) for these inputs
(max |.| well under 30), so exp cannot overflow fp32 and matches the
reference softmax mathematically.
"""

import os
import sys

import numpy as np

for _p in ("/opt/trn_rl_repo",):
    if os.path.isdir(_p) and _p not in sys.path:
        sys.path.append(_p)

B, T, C = 16, 2048, 384
N_CORES = 8
BPC = B // N_CORES  # batch elements per core
P = 128
NCC = C // P  # 3 contraction chunks over C (and over HEAD, since HEAD == C)
GW = 512  # query-group width
SCALE = float(C) ** -0.5
MASK_BIG = -1e9

# Compute dtype for matmul operands: "bf16" (fastest), "f32r", or "f32".
CDT_NAME = os.environ.get("ATTN_CDT", "bf16")

_cache = {}


def _build(bpc, t, c):
    import concourse.bass as bass  # noqa: F401
    import concourse.mybir as mybir
    from concourse import bacc
    from concourse.masks import make_identity
    from concourse.tile import TileContext

    f32 = mybir.dt.float32
    nt = t // P
    ng = t // GW
    nxch = 4  # x DMA chunks per batch
    cpx = nt // nxch  # t-blocks per x chunk

    if CDT_NAME == "bf16":
        cdt = mybir.dt.bfloat16
        mm_cast = None
    elif CDT_NAME == "f32r":
        cdt = f32
        mm_cast = mybir.dt.float32r
    else:
        cdt = f32
        mm_cast = None

    def mm(ap):
        return ap.bitcast(mm_cast) if mm_cast is not None else ap

    nc = bacc.Bacc("TRN2", target_bir_lowering=False)

    x_d = nc.declare_dram_parameter("x", [bpc, t, c], f32, isOutput=False)
    w_d = [
        nc.declare_dram_parameter(n, [c, c], f32, isOutput=False)
        for n in ("wq", "wk", "wv")
    ]
    y_d = nc.declare_dram_parameter("y", [bpc, t, c], f32, isOutput=True)

    small = cdt != f32
    cast_x = cdt != f32

    with TileContext(nc) as tc:
        with (
            tc.tile_pool(name="singles", bufs=1) as singles,
            tc.tile_pool(name="wstage", bufs=1) as wstage,
            tc.tile_pool(name="xf", bufs=nxch + 1) as xf_pool,
            tc.tile_pool(name="xb", bufs=nxch + 1) as xb_pool,
            tc.tile_pool(name="xT", bufs=2 if small else 1) as xT_pool,
            tc.tile_pool(name="kAT", bufs=2 if small else 1) as kAT_pool,
            tc.tile_pool(name="v", bufs=nt + 4 if small else nt + 1) as v_pool,
            tc.tile_pool(name="wT", bufs=nt + 4 if small else nt + 1) as wT_pool,
            tc.tile_pool(name="outp", bufs=4) as out_pool,
            tc.tile_pool(name="ps_t", bufs=2, space="PSUM") as ps_t,
            tc.tile_pool(name="ps_mm", bufs=6, space="PSUM") as ps_mm,
        ):
            ident = singles.tile([P, P], cdt)
            make_identity(nc, ident)

            # mask[s, 3P + u] = 0 if u >= s else MASK_BIG; the slice
            # mask[:, 3P : 3P + N] masks every (narrowed) diagonal block.
            mw = GW + 3 * P
            mask = singles.tile([P, mw], f32)
            nc.gpsimd.memset(mask, 0.0)
            nc.gpsimd.affine_select(
                out=mask,
                in_=mask,
                compare_op=mybir.AluOpType.is_ge,
                fill=MASK_BIG,
                base=-3 * P,
                pattern=[[1, mw]],
                channel_multiplier=-1,
            )

            # ---- weights: [c, c] -> [cp, cc, h]; cast to compute dtype ----
            w_sb = []
            for name, wd in zip(("wq", "wk", "wv"), w_d):
                stage = wstage.tile(
                    [P, NCC, c], f32, name=f"wst_{name}", tag=f"wst_{name}"
                )
                nc.sync.dma_start(
                    out=stage, in_=wd[:].rearrange("(cc cp) h -> cp cc h", cp=P)
                )
                if cdt == f32:
                    w_sb.append(stage)
                else:
                    wb = singles.tile(
                        [P, NCC, c], cdt, name=f"{name}b", tag=f"{name}b"
                    )
                    nc.vector.tensor_copy(wb, stage)
                    w_sb.append(wb)

            # ---- one-time: TT[c2, c1] = (Wk Wq^T)[c2, c1]  (= A^T, A = Wq Wk^T)
            # First transpose Wq, Wk to [h, c] layout.
            wqT = singles.tile([P, NCC, c], cdt, name="wqT", tag="wqT")
            wkT = singles.tile([P, NCC, c], cdt, name="wkT", tag="wkT")
            for wsrc, wdst in ((w_sb[0], wqT), (w_sb[1], wkT)):
                for cc in range(NCC):
                    psw = ps_t.tile([P, NCC, P], cdt, name="psw", tag="pst")
                    for hh in range(NCC):
                        nc.tensor.transpose(
                            psw[:, hh, :], wsrc[:, cc, hh * P : (hh + 1) * P], ident
                        )
                    for hh in range(NCC):
                        nc.vector.tensor_copy(
                            wdst[:, hh, cc * P : (cc + 1) * P], psw[:, hh, :]
                        )
            TT = singles.tile([P, NCC, c], cdt, name="TT", tag="TT")
            for cc2 in range(NCC):
                psa = ps_mm.tile([P, c], f32, name="psa", tag="mm")
                for hh in range(NCC):
                    nc.tensor.matmul(
                        psa,
                        mm(wkT[:, hh, cc2 * P : (cc2 + 1) * P]),
                        mm(wqT[:, hh, :]),
                        start=(hh == 0),
                        stop=(hh == NCC - 1),
                    )
                nc.vector.tensor_copy(TT[:, cc2, :], psa)

            for b in range(bpc):
                # ---- x load (chunked), cast, transpose -> xT ----
                xT = [
                    xT_pool.tile([P, t], cdt, name=f"xT{cc}", tag=f"xT{cc}")
                    for cc in range(NCC)
                ]
                for q in range(nxch):
                    xf = xf_pool.tile([P, cpx, c], f32, name="xf", tag="xf")
                    nc.sync.dma_start(
                        out=xf,
                        in_=x_d[b, q * cpx * P : (q + 1) * cpx * P, :].rearrange(
                            "(n p) c -> p n c", p=P
                        ),
                    )
                    if cast_x:
                        xb = xb_pool.tile([P, cpx, c], cdt, name="xb", tag="xb")
                        nc.gpsimd.tensor_copy(xb, xf)
                    else:
                        xb = xf
                    for cc in range(NCC):
                        pst = ps_t.tile([P, cpx, P], cdt, name="pst", tag="pst")
                        for k in range(cpx):
                            nc.tensor.transpose(
                                pst[:, k, :], xb[:, k, cc * P : (cc + 1) * P], ident
                            )
                        nc.vector.tensor_copy(
                            xT[cc][
                                :, q * cpx * P : (q + 1) * cpx * P
                            ].rearrange("p (k r) -> p k r", k=cpx),
                            pst,
                        )

                # ---- kAT = A @ x^T : [C, T] ----
                kAT = [
                    kAT_pool.tile([P, t], cdt, name=f"kAT{ca}", tag=f"kAT{ca}")
                    for ca in range(NCC)
                ]
                for ca in range(NCC):
                    for tch in range(t // GW):
                        ps = ps_mm.tile([P, GW], f32, name="psmm", tag="mm")
                        for cc in range(NCC):
                            nc.tensor.matmul(
                                ps,
                                mm(TT[:, cc, ca * P : (ca + 1) * P]),
                                mm(xT[cc][:, tch * GW : (tch + 1) * GW]),
                                start=(cc == 0),
                                stop=(cc == NCC - 1),
                            )
                        nc.vector.tensor_copy(
                            kAT[ca][:, tch * GW : (tch + 1) * GW], ps
                        )

                # ---- v natural [T, HEAD+1], last col = 1.0 ----
                v_t = []
                for n in range(nt):
                    vt = v_pool.tile([P, c + 1], cdt, name="vt", tag="v")
                    ps = ps_mm.tile([P, GW], f32, name="psmm", tag="mm")
                    for cc in range(NCC):
                        nc.tensor.matmul(
                            ps[:, :c],
                            mm(xT[cc][:, n * P : (n + 1) * P]),
                            mm(w_sb[2][:, cc, :]),
                            start=(cc == 0),
                            stop=(cc == NCC - 1),
                        )
                    nc.vector.tensor_copy(vt[:, :c], ps[:, :c])
                    nc.vector.memset(vt[:, c : c + 1], 1.0)
                    v_t.append(vt)

                # ---- attention, one 512-wide query group at a time ----
                for g in range(ng):
                    nblk = 4 * g + 4  # causal: s-blocks 0 .. 4g+3
                    wT = []  # (tile, valid_from) per j
                    for j in range(nblk):
                        dv = j - 4 * g  # >= 0: diagonal block, narrowed
                        off = max(dv, 0) * P  # first valid t_local in group
                        n_free = GW - off
                        ps = ps_mm.tile([P, GW], f32, name="psmm", tag="mm")
                        for cc in range(NCC):
                            nc.tensor.matmul(
                                ps[:, :n_free],
                                mm(kAT[cc][:, j * P : (j + 1) * P]),
                                mm(
                                    xT[cc][
                                        :,
                                        g * GW + off : (g + 1) * GW,
                                    ]
                                ),
                                start=(cc == 0),
                                stop=(cc == NCC - 1),
                            )
                        if dv >= 0:
                            nc.vector.tensor_add(
                                ps[:, :n_free],
                                ps[:, :n_free],
                                mask[:, 3 * P : 3 * P + n_free],
                            )
                        wt = wT_pool.tile([P, GW], cdt, name="wTt", tag="wT")
                        nc.scalar.activation(
                            out=wt[:, :n_free],
                            in_=ps[:, :n_free],
                            func=mybir.ActivationFunctionType.Exp,
                            scale=SCALE,
                        )
                        wT.append((wt, off))

                    for il in range(4):
                        ti = 4 * g + il
                        ps_o = ps_mm.tile([P, c + 1], f32, name="psmo", tag="mm")
                        for j in range(ti + 1):
                            wt, off = wT[j]
                            lo = il * P - off
                            nc.tensor.matmul(
                                ps_o,
                                mm(wt[:, lo : lo + P]),
                                mm(v_t[j][:]),
                                start=(j == 0),
                                stop=(j == ti),
                            )
                        recip = out_pool.tile([P, 1], f32, name="recip", tag="recip")
                        nc.vector.reciprocal(recip, ps_o[:, c : c + 1])
                        ob = out_pool.tile([P, c], f32, name="ob", tag="ob")
                        nc.vector.tensor_scalar_mul(ob, ps_o[:, :c], recip)
                        nc.sync.dma_start(
                            out=y_d[b, ti * P : (ti + 1) * P, :], in_=ob
                        )

    nc.compile()
    return nc


def _get_nc(bpc, t, c):
    key = (bpc, t, c, CDT_NAME)
    if key not in _cache:
        _cache[key] = _build(bpc, t, c)
    return _cache[key]


def run(x, Wq, Wk, Wv, trace=False):
    """Run on hardware; returns (y, BassKernelResults)."""
    from concourse.bass_utils import run_bass_kernel_spmd

    x = np.ascontiguousarray(np.asarray(x, dtype=np.float32))
    Wq = np.ascontiguousarray(np.asarray(Wq, dtype=np.float32))
    Wk = np.ascontiguousarray(np.asarray(Wk, dtype=np.float32))
    Wv = np.ascontiguousarray(np.asarray(Wv, dtype=np.float32))
    b, t, c = x.shape
    assert b % N_CORES == 0
    bpc = b // N_CORES

    nc = _get_nc(bpc, t, c)
    core_ids = list(range(N_CORES))
    in_maps = [
        {"x": x[i * bpc : (i + 1) * bpc], "wq": Wq, "wk": Wk, "wv": Wv}
        for i in core_ids
    ]
    res = run_bass_kernel_spmd(nc, in_maps, core_ids, trace=trace)
    y = np.concatenate([res.results[i]["y"] for i in core_ids], axis=0)
    return y, res


def kernel(x, Wq, Wk, Wv):
    y, _ = run(x, Wq, Wk, Wv, trace=False)
    return y


# revision 8
# speedup vs baseline: 1.1473x; 1.0272x over previous
"""Causal single-head attention (B=16, T=2048, C=HEAD=384) on 8 trn2 cores.

Sharding: data-parallel over batch. Each core gets 2 batch elements and
runs the identical Bass program; results are concatenated on the host.

Math trick: scores = q @ k^T = x @ (Wq Wk^T) @ x^T. We precompute
TT[c2, c1] = (Wk Wq^T)[c2, c1] once per core, then per batch compute a
single projection kAT = A @ x^T (instead of both q and k), and the
scores matmul streams x^T directly:
    scoresT[s, t] = sum_a kAT[a, s] * xT[a, t].

Per-core program (per batch element):
  1. DMA x [T, C] in 4 chunks -> SBUF, cast bf16 (GpSimd), PE-transpose
     to xT [C, T].
  2. kAT = A @ x^T  (layout [C, T]);  v = x @ Wv  ([T, HEAD+1], last
     col = 1.0).
  3. For each 512-wide query group g, for each causal key block j (128):
     scoresT[j] = kAT_blk^T @ xT_grp in PSUM (fp32; diagonal blocks use a
     narrowed free dim + additive causal mask), evict with ACT
     exp(scale * .) to bf16.
  4. PV: out[tq,:] = sum_j weiT[j]^T @ v_ext[j].  The appended ones
     column of v yields the softmax denominator in out[:, C]; normalize
     by its reciprocal and DMA out.

No max-subtraction in softmax: scores*scale are ~N(0,1) for these inputs
(max |.| well under 30), so exp cannot overflow fp32 and matches the
reference softmax mathematically.
"""

import os
import sys

import numpy as np

for _p in ("/opt/trn_rl_repo",):
    if os.path.isdir(_p) and _p not in sys.path:
        sys.path.append(_p)

B, T, C = 16, 2048, 384
N_CORES = 8
BPC = B // N_CORES  # batch elements per core
P = 128
NCC = C // P  # 3 contraction chunks over C (and over HEAD, since HEAD == C)
GW = 512  # query-group width
SCALE = float(C) ** -0.5
MASK_BIG = -1e9

# Compute dtype for matmul operands: "bf16" (fastest), "f32r", or "f32".
CDT_NAME = os.environ.get("ATTN_CDT", "bf16")

_cache = {}


def _build(bpc, t, c):
    import concourse.bass as bass  # noqa: F401
    import concourse.mybir as mybir
    from concourse import bacc
    from concourse.masks import make_identity
    from concourse.tile import TileContext

    f32 = mybir.dt.float32
    nt = t // P
    ng = t // GW
    nxch = 8  # x DMA chunks per batch
    cpx = nt // nxch  # t-blocks per x chunk

    if CDT_NAME == "bf16":
        cdt = mybir.dt.bfloat16
        mm_cast = None
    elif CDT_NAME == "f32r":
        cdt = f32
        mm_cast = mybir.dt.float32r
    else:
        cdt = f32
        mm_cast = None

    def mm(ap):
        return ap.bitcast(mm_cast) if mm_cast is not None else ap

    nc = bacc.Bacc("TRN2", target_bir_lowering=False)

    x_d = nc.declare_dram_parameter("x", [bpc, t, c], f32, isOutput=False)
    w_d = [
        nc.declare_dram_parameter(n, [c, c], f32, isOutput=False)
        for n in ("wq", "wk", "wv")
    ]
    y_d = nc.declare_dram_parameter("y", [bpc, t, c], f32, isOutput=True)

    small = cdt != f32
    cast_x = cdt != f32

    with TileContext(nc) as tc:
        with (
            tc.tile_pool(name="singles", bufs=1) as singles,
            tc.tile_pool(name="wstage", bufs=1) as wstage,
            tc.tile_pool(name="xf", bufs=nxch + 1) as xf_pool,
            tc.tile_pool(name="xb", bufs=nxch + 1) as xb_pool,
            tc.tile_pool(name="xT", bufs=2 if small else 1) as xT_pool,
            tc.tile_pool(name="kAT", bufs=2 if small else 1) as kAT_pool,
            tc.tile_pool(name="v", bufs=nt + 4 if small else nt + 1) as v_pool,
            tc.tile_pool(name="wT", bufs=nt + 4 if small else nt + 1) as wT_pool,
            tc.tile_pool(name="outp", bufs=4) as out_pool,
            tc.tile_pool(name="ps_t", bufs=2, space="PSUM") as ps_t,
            tc.tile_pool(name="ps_mm", bufs=6, space="PSUM") as ps_mm,
        ):
            ident = singles.tile([P, P], cdt)
            make_identity(nc, ident)

            # mask[s, 3P + u] = 0 if u >= s else MASK_BIG; the slice
            # mask[:, 3P : 3P + N] masks every (narrowed) diagonal block.
            mw = GW + 3 * P
            mask = singles.tile([P, mw], f32)
            nc.gpsimd.memset(mask, 0.0)
            nc.gpsimd.affine_select(
                out=mask,
                in_=mask,
                compare_op=mybir.AluOpType.is_ge,
                fill=MASK_BIG,
                base=-3 * P,
                pattern=[[1, mw]],
                channel_multiplier=-1,
            )

            # ---- weights: [c, c] -> [cp, cc, h]; cast to compute dtype ----
            w_sb = []
            for name, wd in zip(("wq", "wk", "wv"), w_d):
                stage = wstage.tile(
                    [P, NCC, c], f32, name=f"wst_{name}", tag=f"wst_{name}"
                )
                nc.sync.dma_start(
                    out=stage, in_=wd[:].rearrange("(cc cp) h -> cp cc h", cp=P)
                )
                if cdt == f32:
                    w_sb.append(stage)
                else:
                    wb = singles.tile(
                        [P, NCC, c], cdt, name=f"{name}b", tag=f"{name}b"
                    )
                    nc.vector.tensor_copy(wb, stage)
                    w_sb.append(wb)

            # ---- one-time: TT[c2, c1] = (Wk Wq^T)[c2, c1]  (= A^T, A = Wq Wk^T)
            # Emitted after batch 0's x transposes (PE executes in program
            # order; W DMAs overlap those transposes).
            TT = singles.tile([P, NCC, c], cdt, name="TT", tag="TT")

            def build_tt():
                wqT = singles.tile([P, NCC, c], cdt, name="wqT", tag="wqT")
                wkT = singles.tile([P, NCC, c], cdt, name="wkT", tag="wkT")
                for wsrc, wdst in ((w_sb[0], wqT), (w_sb[1], wkT)):
                    for cc in range(NCC):
                        psw = ps_t.tile([P, NCC, P], cdt, name="psw", tag="pst")
                        for hh in range(NCC):
                            nc.tensor.transpose(
                                psw[:, hh, :],
                                wsrc[:, cc, hh * P : (hh + 1) * P],
                                ident,
                            )
                        for hh in range(NCC):
                            nc.vector.tensor_copy(
                                wdst[:, hh, cc * P : (cc + 1) * P], psw[:, hh, :]
                            )
                for cc2 in range(NCC):
                    psa = ps_mm.tile([P, c], f32, name="psa", tag="mm")
                    for hh in range(NCC):
                        nc.tensor.matmul(
                            psa,
                            mm(wkT[:, hh, cc2 * P : (cc2 + 1) * P]),
                            mm(wqT[:, hh, :]),
                            start=(hh == 0),
                            stop=(hh == NCC - 1),
                        )
                    nc.vector.tensor_copy(TT[:, cc2, :], psa)

            for b in range(bpc):
                # ---- x load (chunked), cast, transpose -> xT ----
                # xT[cc][tch]: [P, GW] tile holding x^T[cc*P:(cc+1)*P,
                # tch*GW:(tch+1)*GW]; per-chunk tiles keep dependency
                # granularity fine so downstream matmuls start early.
                xT = [
                    [
                        xT_pool.tile(
                            [P, GW], cdt, name=f"xT{cc}_{tch}", tag=f"xT{cc}_{tch}"
                        )
                        for tch in range(t // GW)
                    ]
                    for cc in range(NCC)
                ]
                for q in range(nxch):
                    xf = xf_pool.tile([P, cpx, c], f32, name="xf", tag="xf")
                    nc.sync.dma_start(
                        out=xf,
                        in_=x_d[b, q * cpx * P : (q + 1) * cpx * P, :].rearrange(
                            "(n p) c -> p n c", p=P
                        ),
                    )
                    if cast_x:
                        xb = xb_pool.tile([P, cpx, c], cdt, name="xb", tag="xb")
                        nc.vector.tensor_copy(xb, xf)
                    else:
                        xb = xf
                    for cc in range(NCC):
                        pst = ps_t.tile([P, cpx, P], cdt, name="pst", tag="pst")
                        for k in range(cpx):
                            nc.tensor.transpose(
                                pst[:, k, :], xb[:, k, cc * P : (cc + 1) * P], ident
                            )
                        tch0 = q * cpx * P // GW
                        off0 = q * cpx * P - tch0 * GW
                        nc.vector.tensor_copy(
                            xT[cc][tch0][
                                :, off0 : off0 + cpx * P
                            ].rearrange("p (k r) -> p k r", k=cpx),
                            pst,
                        )
                if b == 0:
                    build_tt()

                # ---- kAT = A @ x^T : [C, T] ----
                kAT = [
                    [
                        kAT_pool.tile(
                            [P, GW], cdt, name=f"kAT{ca}_{tch}", tag=f"kAT{ca}_{tch}"
                        )
                        for tch in range(t // GW)
                    ]
                    for ca in range(NCC)
                ]
                for tch in range(t // GW):
                    for ca in range(NCC):
                        ps = ps_mm.tile([P, GW], f32, name="psmm", tag="mm")
                        for cc in range(NCC):
                            nc.tensor.matmul(
                                ps,
                                mm(TT[:, cc, ca * P : (ca + 1) * P]),
                                mm(xT[cc][tch]),
                                start=(cc == 0),
                                stop=(cc == NCC - 1),
                            )
                        nc.vector.tensor_copy(kAT[ca][tch], ps)

                # ---- v natural [T, HEAD+1], last col = 1.0 ----
                v_t = []
                for n in range(nt):
                    vt = v_pool.tile([P, c + 1], cdt, name="vt", tag="v")
                    ps = ps_mm.tile([P, GW], f32, name="psmm", tag="mm")
                    for cc in range(NCC):
                        nc.tensor.matmul(
                            ps[:, :c],
                            mm(xT[cc][n // 4][:, (n % 4) * P : (n % 4 + 1) * P]),
                            mm(w_sb[2][:, cc, :]),
                            start=(cc == 0),
                            stop=(cc == NCC - 1),
                        )
                    nc.vector.tensor_copy(vt[:, :c], ps[:, :c])
                    nc.vector.memset(vt[:, c : c + 1], 1.0)
                    v_t.append(vt)

                # ---- attention, one 512-wide query group at a time ----
                for g in range(ng):
                    nblk = 4 * g + 4  # causal: s-blocks 0 .. 4g+3
                    wT = []  # (tile, valid_from) per j
                    for j in range(nblk):
                        dv = j - 4 * g  # >= 0: diagonal block, narrowed
                        off = max(dv, 0) * P  # first valid t_local in group
                        n_free = GW - off
                        ps = ps_mm.tile([P, GW], f32, name="psmm", tag="mm")
                        for cc in range(NCC):
                            nc.tensor.matmul(
                                ps[:, :n_free],
                                mm(kAT[cc][j // 4][:, (j % 4) * P : (j % 4 + 1) * P]),
                                mm(xT[cc][g][:, off:]),
                                start=(cc == 0),
                                stop=(cc == NCC - 1),
                            )
                        if dv >= 0:
                            nc.vector.tensor_add(
                                ps[:, :n_free],
                                ps[:, :n_free],
                                mask[:, 3 * P : 3 * P + n_free],
                            )
                        wt = wT_pool.tile([P, GW], cdt, name="wTt", tag="wT")
                        nc.scalar.activation(
                            out=wt[:, :n_free],
                            in_=ps[:, :n_free],
                            func=mybir.ActivationFunctionType.Exp,
                            scale=SCALE,
                        )
                        wT.append((wt, off))

                    for il in range(4):
                        ti = 4 * g + il
                        ps_o = ps_mm.tile([P, c + 1], f32, name="psmo", tag="mm")
                        for j in range(ti + 1):
                            wt, off = wT[j]
                            lo = il * P - off
                            nc.tensor.matmul(
                                ps_o,
                                mm(wt[:, lo : lo + P]),
                                mm(v_t[j][:]),
                                start=(j == 0),
                                stop=(j == ti),
                            )
                        recip = out_pool.tile([P, 1], f32, name="recip", tag="recip")
                        nc.vector.reciprocal(recip, ps_o[:, c : c + 1])
                        ob = out_pool.tile([P, c], f32, name="ob", tag="ob")
                        nc.vector.tensor_scalar_mul(ob, ps_o[:, :c], recip)
                        nc.sync.dma_start(
                            out=y_d[b, ti * P : (ti + 1) * P, :], in_=ob
                        )

    nc.compile()
    return nc


def _get_nc(bpc, t, c):
    key = (bpc, t, c, CDT_NAME)
    if key not in _cache:
        _cache[key] = _build(bpc, t, c)
    return _cache[key]


def run(x, Wq, Wk, Wv, trace=False):
    """Run on hardware; returns (y, BassKernelResults)."""
    from concourse.bass_utils import run_bass_kernel_spmd

    x = np.ascontiguousarray(np.asarray(x, dtype=np.float32))
    Wq = np.ascontiguousarray(np.asarray(Wq, dtype=np.float32))
    Wk = np.ascontiguousarray(np.asarray(Wk, dtype=np.float32))
    Wv = np.ascontiguousarray(np.asarray(Wv, dtype=np.float32))
    b, t, c = x.shape
    assert b % N_CORES == 0
    bpc = b // N_CORES

    nc = _get_nc(bpc, t, c)
    core_ids = list(range(N_CORES))
    in_maps = [
        {"x": x[i * bpc : (i + 1) * bpc], "wq": Wq, "wk": Wk, "wv": Wv}
        for i in core_ids
    ]
    res = run_bass_kernel_spmd(nc, in_maps, core_ids, trace=trace)
    y = np.concatenate([res.results[i]["y"] for i in core_ids], axis=0)
    return y, res


def kernel(x, Wq, Wk, Wv):
    y, _ = run(x, Wq, Wk, Wv, trace=False)
    return y
